# revision 16
# baseline (speedup 1.0000x reference)
"""Trainium2 Bass kernel for nn_BoundaryConditionedFusion.

Sharding: 8 cores = 2 batches x 4 query-blocks of 1024 pixels (16 image rows).
Each core computes k/v/b_weight for its full batch (cheap 1x1 convs) and
flash-style attention + proj + 3x3 local conv only for its query block.
All BatchNorms are folded into weights on the host; per-core inputs carry the
query-block-specific slices (with halo rows) so one SPMD program serves all
cores. Host reassembles the 8 [64, 1024] output slices.

Attention per (head, q-block): T = K^T Q computed k-major in [128, 512] PSUM
chunks (4 per group via PE row-tiling at tile_position (32g, 0) with q/k
replicated at partition offsets 0/32/64/96), exp over the 4-bank group in one
ScalarE op, then PV accumulated with an M=33 stationary (8 v dims + ones col
at partition 32) so out^T and the softmax denominator fall out of one matmul
chain. Softmax max-subtraction is skipped: logits are bounded (|T| < 9).
"""
import sys
import types
import numpy as np

C = 64
HEADS = 8
HD = 8
EPS = 1e-5
N = 4096
SCALE = HD ** -0.5
QBLK = 1024
NCORES = 8

# ---------------------------------------------------------------- compat shims


def _install_ntff_shim():
    try:
        from antenv import axon_hooks  # noqa: F401
        return
    except ImportError:
        pass
    try:
        from trn_agent_boot.trn_boot import _ntff_profile_via_ctypes
        hook = _ntff_profile_via_ctypes('/opt/axon/libaxon_pjrt.so')
    except Exception:
        return
    mod = types.ModuleType("antenv.axon_hooks")
    mod._hook = hook
    mod.get_axon_ntff_profile_hook = lambda: mod._hook
    mod.set_axon_ntff_profile_hook = lambda h: setattr(mod, "_hook", h)
    sys.modules["antenv.axon_hooks"] = mod
    import antenv
    antenv.axon_hooks = mod


def _fix_multiwait(raw: bytes) -> bytes:
    """This walrus build allows one sync-wait per instruction; hoist extras
    onto preceding same-engine NoOps."""
    import orjson
    m = orjson.loads(raw)

    def fix_block(insts):
        out = []
        for inst in insts:
            si = inst.get("sync_info") or {}
            waits = si.get("on_wait") or []
            if len(waits) > 1:
                for j, w in enumerate(waits[:-1]):
                    out.append({
                        "debug": inst.get("debug", 0),
                        "engine": inst["engine"],
                        "ins": [], "outs": [],
                        "name": f"{inst['name']}_w{j}",
                        "opcode": "NoOp",
                        "sync_info": {"on_update": [], "on_wait": [w]},
                    })
                inst = dict(inst)
                inst["sync_info"] = {"on_update": si.get("on_update") or [],
                                     "on_wait": [waits[-1]]}
            out.append(inst)
        return out

    def walk(o):
        if isinstance(o, dict):
            if isinstance(o.get("instructions"), list):
                o["instructions"] = fix_block(o["instructions"])
            for v in o.values():
                walk(v)
        elif isinstance(o, list):
            for v in o:
                walk(v)

    walk(m)
    return orjson.dumps(m)


# ---------------------------------------------------------------- host prep


def _resize_matrix(n_out, n_in):
    """Row matrix of jax.image.resize(..., 'bilinear', antialias=True)."""
    R = np.zeros((n_out, n_in), np.float64)
    scale = n_in / n_out
    for i in range(n_out):
        center = (i + 0.5) * scale - 0.5
        lo = int(np.floor(center - scale))
        hi = int(np.ceil(center + scale))
        ws, js = [], []
        for j in range(max(lo, 0), min(hi + 1, n_in)):
            w = max(0.0, 1.0 - abs(j - center) / scale)
            if w > 0:
                js.append(j)
                ws.append(w)
        ws = np.array(ws)
        ws /= ws.sum()
        for j, w in zip(js, ws):
            R[i, j] += w
    return R.astype(np.float32)


def _fold_bn(p):
    gamma, beta, mean, var = [np.asarray(t, np.float32) for t in p]
    s = gamma / np.sqrt(var + EPS)
    return s, beta - mean * s


def _prep_weights(params):
    pr = {}
    s_in, sh_in = _fold_bn(params['norm_in'])
    Wqkv = np.asarray(params['qkv_w'], np.float32)
    Wf = Wqkv * s_in[None, :]
    cqkv = Wqkv @ sh_in
    pr['wq'] = np.ascontiguousarray(Wf[:64].T).astype(np.float16)   # [128, 64]
    pr['wk'] = np.ascontiguousarray(Wf[64:128].T).astype(np.float16)
    pr['wv'] = np.ascontiguousarray(Wf[128:].T).astype(np.float16)
    pr['cq'] = cqkv[:64].reshape(64, 1).copy()
    pr['ck'] = cqkv[64:128].reshape(64, 1).copy()
    cv = cqkv[128:].copy()
    s_b, sh_b = _fold_bn(params['bproj_bn'])
    Wb = np.asarray(params['bproj_w'], np.float32)[:, 0] * s_b[:, None, None]
    pr['wb'] = np.ascontiguousarray(Wb.reshape(64, 9).T).astype(np.float16)  # [9, 64]
    pr['cb'] = (np.asarray(params['bproj_b'], np.float32) * s_b + sh_b).reshape(64, 1).copy()
    s_p, sh_p = _fold_bn(params['norm'])
    Wp = np.asarray(params['proj_w'], np.float32) * s_p[:, None]
    bp = np.asarray(params['proj_b'], np.float32) * s_p + sh_p
    pr['wp'] = np.ascontiguousarray(Wp.T).astype(np.float16)        # [64, 64]
    s_l, sh_l = _fold_bn(params['local_bn'])
    Wl = np.asarray(params['local_w'], np.float32) * s_l[:, None, None, None]
    bl = np.asarray(params['local_b'], np.float32) * s_l + sh_l
    pr['wl'] = np.ascontiguousarray(
        Wl.reshape(64, 128, 9).transpose(1, 2, 0).reshape(128, 9 * 64))
    pr['bf'] = (bl + bp + Wp @ cv).reshape(64, 1).copy()
    R = _resize_matrix(64, 128)
    pr['rT'] = np.ascontiguousarray(R.T)                  # [128, 64]
    expd = np.zeros((16, 128), np.float32)
    for i in range(16):
        expd[i, 8 * i:8 * i + 8] = 1.0
    pr['expd'] = expd
    Rpad = np.zeros((66, 134), np.float32)
    Rpad[1:65, 3:131] = R
    pr['rqT'] = []
    for qb in range(4):
        r0 = qb * 16
        # b_down rows r0-1..r0+16 over bm rows 2*r0-3..2*r0+34 (pad offset 1 / 3)
        Rq = Rpad[r0:r0 + 18, 2 * r0:2 * r0 + 38]         # [18, 38]
        pr['rqT'].append(np.ascontiguousarray(Rq.T))      # [38, 18]
    return pr


def _build_in_maps(f1, f2, bm, pr):
    f1 = np.asarray(f1, np.float32).reshape(2, 64, N)
    f2 = np.asarray(f2, np.float32).reshape(2, 64, N)
    bm = np.asarray(bm, np.float32).reshape(2, 128, 128)
    bmpad = np.zeros((2, 134, 128), np.float32)
    bmpad[:, 3:131, :] = bm
    maps = []
    for core in range(NCORES):
        b, qb = core // 4, core % 4
        r0 = qb * 16
        xq = np.zeros((128, 18, 64), np.float32)
        lo, hi = max(0, r0 - 1), min(64, r0 + 17)
        xq[:64, lo - (r0 - 1):hi - (r0 - 1), :] = f1[b].reshape(64, 64, 64)[:, lo:hi, :]
        xq[64:, lo - (r0 - 1):hi - (r0 - 1), :] = f2[b].reshape(64, 64, 64)[:, lo:hi, :]
        m = {
            "xall1": np.ascontiguousarray(f1[b]).astype(np.float16),
            "xall2": np.ascontiguousarray(f2[b]).astype(np.float16),
            "xq": np.ascontiguousarray(xq.reshape(128, 18 * 64)).astype(np.float16),
            "xqf": np.ascontiguousarray(xq.reshape(128, 18 * 64)),
            "bm": np.ascontiguousarray(bm[b]),
            "bmq": np.ascontiguousarray(bmpad[b, 2 * r0:2 * r0 + 38, :]),
            "rqT": pr['rqT'][qb],
        }
        for nm in ("wq", "wk", "wv", "cq", "ck", "wb", "cb", "wp", "wl", "bf", "rT",
                   "expd"):
            m[nm] = pr[nm]
        maps.append(m)
    return maps


# ---------------------------------------------------------------- device program


def _build_program():
    import concourse.bass as bass
    import concourse.mybir as mybir
    from concourse.tile import TileContext

    F32 = mybir.dt.float32
    F16 = mybir.dt.float16
    AF = mybir.ActivationFunctionType
    taps = [(dr, dc) for dr in (-1, 0, 1) for dc in (-1, 0, 1)]

    nc = bass.Bass()
    di = lambda nm, shp, dt=F32: nc.declare_dram_parameter(nm, shp, dt, isOutput=False)
    xall1 = di("xall1", [64, N], F16)
    xall2 = di("xall2", [64, N], F16)
    xq_d = di("xq", [128, 18 * 64], F16)
    xqf_d = di("xqf", [128, 18 * 64])
    bm_d = di("bm", [128, 128])
    bmq_d = di("bmq", [38, 128])
    rT_d = di("rT", [128, 64])
    rqT_d = di("rqT", [38, 18])
    wq_d = di("wq", [128, 64], F16)
    wk_d = di("wk", [128, 64], F16)
    wv_d = di("wv", [128, 64], F16)
    cq_d = di("cq", [64, 1])
    ck_d = di("ck", [64, 1])
    wb_d = di("wb", [9, 64], F16)
    cb_d = di("cb", [64, 1])
    wp_d = di("wp", [64, 64], F16)
    wl_d = di("wl", [128, 9 * 64])
    bf_d = di("bf", [64, 1])
    expd_d = di("expd", [16, 128])
    out_d = nc.declare_dram_parameter("out", [64, QBLK], F32, isOutput=True)

    with TileContext(nc) as tc:
        with (
            tc.tile_pool(name="w", bufs=1) as wpool,
            tc.tile_pool(name="big", bufs=1) as big,
            tc.tile_pool(name="asm", bufs=2) as asm,
        ):
            wp_s = wpool.tile([128, 64], F16)
            nc.sync.dma_start(out=wp_s[0:64, :], in_=wp_d[:])
            nc.sync.dma_start(out=wp_s[64:128, :], in_=wp_d[:])
            wl_s = wpool.tile([128, 9 * 64], F32); nc.sync.dma_start(out=wl_s[:], in_=wl_d[:])
            bf_s = wpool.tile([64, 1], F32);  nc.sync.dma_start(out=bf_s[:], in_=bf_d[:])
            expd_s = wpool.tile([16, 128], F32)
            nc.sync.dma_start(out=expd_s[:], in_=expd_d[:])

            xq_pad = big.tile([128, 18 * 66], F16)
            xq3 = xq_pad[:].rearrange("p (r c) -> p r c", c=66)
            nc.vector.memset(xq_pad[:].rearrange("p (r c) -> p (r) c", c=66)[:, :, 65:66], 0.0)
            nc.vector.memset(xq3[:, :, 0:1], 0.0)
            nc.sync.dma_start(out=xq3[:, :, 1:65],
                              in_=xq_d[:].rearrange("p (r c) -> p r c", c=64))
            xq_padF = big.tile([128, 18 * 66], F32)
            xqF3 = xq_padF[:].rearrange("p (r c) -> p r c", c=66)
            nc.vector.memset(xqF3[:, :, 0:1], 0.0)
            nc.vector.memset(xqF3[:, :, 65:66], 0.0)
            nc.sync.dma_start(out=xqF3[:, :, 1:65],
                              in_=xqf_d[:].rearrange("p (r c) -> p r c", c=64))
            k16 = big.tile([64, N], F16)
            q16 = big.tile([64, QBLK], F16)
            v_all = big.tile([128, 32 * 264], F16)

            # ---------------- fringe
            with (
                tc.tile_pool(name="fr", bufs=1) as fr,
                tc.tile_pool(name="ftmp", bufs=2) as ftmp,
                tc.tile_pool(name="fps", bufs=2, space="PSUM") as fps,
            ):
                wq_s = fr.tile([128, 64], F16); nc.sync.dma_start(out=wq_s[:], in_=wq_d[:])
                wk_s = fr.tile([128, 64], F16); nc.sync.dma_start(out=wk_s[:], in_=wk_d[:])
                wv_s = fr.tile([128, 64], F16); nc.sync.dma_start(out=wv_s[:], in_=wv_d[:])
                cq_s = fr.tile([64, 1], F32);  nc.sync.dma_start(out=cq_s[:], in_=cq_d[:])
                ck_s = fr.tile([64, 1], F32);  nc.sync.dma_start(out=ck_s[:], in_=ck_d[:])
                wb_s = fr.tile([9, 64], F16);  nc.gpsimd.dma_start(out=wb_s[:], in_=wb_d[:])
                cb_s = fr.tile([64, 1], F32);  nc.gpsimd.dma_start(out=cb_s[:], in_=cb_d[:])
                rT_s = fr.tile([128, 64], F32); nc.gpsimd.dma_start(out=rT_s[:], in_=rT_d[:])
                rqT_s = fr.tile([38, 18], F32); nc.gpsimd.dma_start(out=rqT_s[:], in_=rqT_d[:])
                bm_s = fr.tile([128, 128], F32); nc.gpsimd.dma_start(out=bm_s[:], in_=bm_d[:])
                bmq_s = fr.tile([38, 128], F32); nc.gpsimd.dma_start(out=bmq_s[:], in_=bmq_d[:])

                x_cm = fr.tile([128, N], F16)
                nc.sync.dma_start(out=x_cm[0:64, :], in_=xall1[:])
                nc.sync.dma_start(out=x_cm[64:128, :], in_=xall2[:])
                xq_cm = fr.tile([128, QBLK], F16)
                nc.sync.dma_start(out=xq_cm[:], in_=xq_d[:, 64:64 + QBLK])

                # bilinear resize (fp32 PE), outputs cast to fp16
                psu = fps.tile([128, 64], F32, tag="fp")
                nc.tensor.matmul(psu[:], bm_s[:], rT_s[:], start=True, stop=True)
                u_sb = ftmp.tile([128, 64], F32, tag="t_r")
                nc.vector.tensor_copy(u_sb[:], psu[:])
                psbd = fps.tile([64, 64], F32, tag="fp")
                nc.tensor.matmul(psbd[:], u_sb[:], rT_s[:], start=True, stop=True)
                bdown = fr.tile([64, 64], F16)
                nc.vector.tensor_copy(bdown[:], psbd[:])
                pstq = fps.tile([128, 18], F32, tag="fp")
                nc.tensor.matmul(pstq[:], bmq_s[:], rqT_s[:], start=True, stop=True)
                tq_sb = ftmp.tile([128, 18], F32, tag="t_r")
                nc.vector.tensor_copy(tq_sb[:], pstq[:])
                psbdq = fps.tile([18, 64], F32, tag="fp")
                nc.tensor.matmul(psbdq[:], tq_sb[:], rT_s[:], start=True, stop=True)
                bdownq = fr.tile([18, 64], F16)
                nc.vector.tensor_copy(bdownq[:], psbdq[:])

                patches = fr.tile([9, N], F16)
                nc.vector.memset(patches[:], 0.0)
                patches_q = fr.tile([9, QBLK], F16)
                nc.vector.memset(patches_q[:], 0.0)
                for t, (dr, dc) in enumerate(taps):
                    rs, re = max(0, -dr), min(64, 64 - dr)
                    cs, ce = max(0, -dc), min(64, 64 - dc)
                    eng = nc.sync if t % 2 == 0 else nc.gpsimd
                    dstp = patches[t:t + 1, :].rearrange("p (r c) -> p r c", c=64)
                    eng.dma_start(out=dstp[:, rs:re, cs:ce],
                                  in_=bdown[rs + dr:re + dr, cs + dc:ce + dc])
                    dstpq = patches_q[t:t + 1, :].rearrange("p (r c) -> p r c", c=64)
                    eng.dma_start(out=dstpq[:, 0:16, cs:ce],
                                  in_=bdownq[1 + dr:17 + dr, cs + dc:ce + dc])

                bwp = fr.tile([64, N], F32)
                for blk in range(8):
                    psb = fps.tile([64, 512], F32, tag="fp")
                    nc.tensor.matmul(psb[:], wb_s[:],
                                     patches[:, blk * 512:(blk + 1) * 512],
                                     start=True, stop=True)
                    sg = ftmp.tile([64, 512], F32, tag="sg")
                    nc.scalar.activation(sg[:], psb[:], AF.Sigmoid, bias=cb_s[:])
                    nc.vector.tensor_scalar_add(bwp[:, blk * 512:(blk + 1) * 512],
                                                sg[:], 1.0)
                bwq = fr.tile([64, QBLK], F32)
                for blk in range(2):
                    psb = fps.tile([64, 512], F32, tag="fp")
                    nc.tensor.matmul(psb[:], wb_s[:],
                                     patches_q[:, blk * 512:(blk + 1) * 512],
                                     start=True, stop=True)
                    sg = ftmp.tile([64, 512], F32, tag="sg")
                    nc.scalar.activation(sg[:], psb[:], AF.Sigmoid, bias=cb_s[:])
                    nc.vector.tensor_scalar(bwq[:, blk * 512:(blk + 1) * 512], sg[:],
                                            1.0, SCALE, mybir.AluOpType.add,
                                            mybir.AluOpType.mult)

                # k16 / q16
                for blk in range(8):
                    psk = fps.tile([64, 512], F32, tag="fp")
                    nc.tensor.matmul(psk[:], wk_s[:],
                                     x_cm[:, blk * 512:(blk + 1) * 512],
                                     start=True, stop=True)
                    kt = ftmp.tile([64, 512], F32, tag="kt")
                    nc.vector.tensor_scalar_add(kt[:], psk[:], ck_s[:])
                    nc.vector.tensor_mul(k16[:, blk * 512:(blk + 1) * 512], kt[:],
                                         bwp[:, blk * 512:(blk + 1) * 512])
                for blk in range(2):
                    psq = fps.tile([64, 512], F32, tag="fp")
                    nc.tensor.matmul(psq[:], wq_s[:],
                                     xq_cm[:, blk * 512:(blk + 1) * 512],
                                     start=True, stop=True)
                    qt = ftmp.tile([64, 512], F32, tag="kt")
                    nc.vector.tensor_scalar_add(qt[:], psq[:], cq_s[:])
                    nc.vector.tensor_mul(q16[:, blk * 512:(blk + 1) * 512], qt[:],
                                         bwq[:, blk * 512:(blk + 1) * 512])

                # v pixel-major (ones col memset only; cols 8..31 left as garbage,
                # their psum rows 8..31 are never read)
                onesv = v_all[:].rearrange("p (ch d) -> p ch d", d=33)
                nc.vector.memset(onesv[:, :, 32:33], 1.0)
                for c in range(32):
                    psv = fps.tile([128, 64], F32, tag="fp")
                    nc.tensor.matmul(psv[:], x_cm[:, c * 128:(c + 1) * 128], wv_s[:],
                                     start=True, stop=True)
                    dstv = v_all[:, c * 264:(c + 1) * 264].rearrange(
                        "p (h d) -> p h d", d=33)
                    nc.vector.tensor_copy(dstv[:, :, 0:8],
                                          psv[:].rearrange("p (h d) -> p h d", d=8))

            # ---------------- attention (fp16, m2 lagged 2 groups)
            GROUPS = [4, 2, 4, 2, 4, 2, 4, 2, 4, 2, 2]
            GSTART = [sum(GROUPS[:i]) for i in range(len(GROUPS))]
            NG = len(GROUPS)
            with (
                tc.tile_pool(name="arep", bufs=2) as arep,
                tc.tile_pool(name="aE", bufs=3) as aE,
                tc.tile_pool(name="apsA", bufs=1, space="PSUM") as apsA,
                tc.tile_pool(name="apsB", bufs=1, space="PSUM") as apsB,
                tc.tile_pool(name="apsO", bufs=2, space="PSUM") as apsO,
            ):
                U_all = asm.tile([128, 512], F32, tag="uall")
                den_all = asm.tile([16, 512], F32, tag="dall")
                for h in range(HEADS):
                    k_rep = arep.tile([128, N], F16, tag="krep")
                    q_rep = arep.tile([128, QBLK], F16, tag="qrep")
                    for g in range(4):
                        nc.gpsimd.dma_start(out=k_rep[32 * g:32 * g + 8, :],
                                            in_=k16[8 * h:8 * h + 8, :])
                        nc.gpsimd.dma_start(out=q_rep[32 * g:32 * g + 8, :],
                                            in_=q16[8 * h:8 * h + 8, :])
                    for qb in range(2):
                        psO = apsO.tile([128, 512], F32, tag="psO")
                        Es = []
                        def m2_group(gi):
                            ng, st = GROUPS[gi], GSTART[gi]
                            for g in range(ng):
                                c = st + g
                                nc.tensor.matmul(
                                    psO[0:33, :],
                                    v_all[:, c * 264 + 33 * h:c * 264 + 33 * h + 33],
                                    Es[gi][:, g * 512:(g + 1) * 512],
                                    start=(c == 0), stop=(c == 31))
                        for gi, ng in enumerate(GROUPS):
                            st = GSTART[gi]
                            pool = apsA if gi % 2 == 0 else apsB
                            psT = pool.tile([128, 512 * ng], F32,
                                            tag="psTA" if gi % 2 == 0 else "psTB")
                            for g in range(ng):
                                c = st + g
                                nc.tensor.matmul(
                                    psT[:, g * 512:(g + 1) * 512],
                                    k_rep[32 * g:32 * g + 8, c * 128:(c + 1) * 128],
                                    q_rep[32 * g:32 * g + 8, qb * 512:(qb + 1) * 512],
                                    start=True, stop=True,
                                    tile_position=(32 * g, 0))
                            E = aE.tile([128, 512 * 4], F16, tag="E")
                            nc.scalar.activation(E[:, :512 * ng], psT[:], AF.Exp)
                            Es.append(E)
                            if gi >= 2:
                                m2_group(gi - 2)
                        m2_group(NG - 2)
                        m2_group(NG - 1)
                        i = qb * 8 + h
                        stage = asm.tile([33, 512], F32, tag="stage")
                        nc.vector.tensor_copy(stage[:], psO[0:33, :])
                        nc.sync.dma_start(out=den_all[i:i + 1, :], in_=stage[32:33, :])
                        nc.sync.dma_start(out=U_all[8 * i:8 * i + 8, :],
                                          in_=stage[0:8, :])

            # ---------------- tail: local conv overlaps normalize, then proj+relu
            with (
                tc.tile_pool(name="fin", bufs=2) as fin,
                tc.tile_pool(name="shf", bufs=1) as shf,
                tc.tile_pool(name="ops", bufs=2, space="PSUM") as ops,
            ):
                shifts = []
                for t, (dr, dc) in enumerate(taps):
                    sh = shf.tile([128, QBLK], F32, tag=f"sh{t}")
                    eng = nc.sync if t % 2 == 0 else nc.gpsimd
                    eng.dma_start(out=sh[:],
                                  in_=xqF3[:, 1 + dr:17 + dr, 1 + dc:65 + dc])
                    shifts.append(sh)
                psfs = []
                for qb in range(2):
                    psf = ops.tile([64, 512], F32, tag="psf")
                    for t in range(9):
                        nc.tensor.matmul(psf[:], wl_s[:, t * 64:(t + 1) * 64],
                                         shifts[t][:, qb * 512:(qb + 1) * 512],
                                         start=(t == 0), stop=False)
                    psfs.append(psf)
                rden_all = asm.tile([16, 512], F32, tag="rdall")
                nc.vector.reciprocal(rden_all[:], den_all[:])
                psB = ops.tile([128, 512], F32, tag="psB")
                nc.tensor.matmul(psB[:], expd_s[:], rden_all[:], start=True, stop=True)
                att16 = asm.tile([128, 512], F16, tag="att16")
                nc.vector.tensor_mul(att16[:], U_all[:], psB[:])
                for qb in range(2):
                    nc.tensor.matmul(psfs[qb][:], wp_s[64 * qb:64 * qb + 64, :],
                                     att16[64 * qb:64 * qb + 64, :],
                                     start=False, stop=True,
                                     tile_position=(64 * qb, 0))
                    o_sb = fin.tile([64, 512], F32, tag="osb")
                    nc.scalar.activation(o_sb[:], psfs[qb][:], AF.Relu, bias=bf_s[:])
                    nc.sync.dma_start(out=out_d[:, qb * 512:(qb + 1) * 512],
                                      in_=o_sb[:])

    return nc


_CACHED = {}


def kernel(f1, f2, boundary_map, params, _profile=False):
    _install_ntff_shim()
    from concourse.bass_utils import run_bass_kernel_spmd

    if "nc" not in _CACHED:
        nc = _build_program()
        orig = nc.to_json_bytes
        nc.to_json_bytes = lambda *a, **k: _fix_multiwait(orig(*a, **k))
        _CACHED["nc"] = nc
    nc = _CACHED["nc"]

    pr = _prep_weights(params)
    in_maps = _build_in_maps(f1, f2, boundary_map, pr)
    res = run_bass_kernel_spmd(nc, in_maps, core_ids=list(range(NCORES)),
                               trace=bool(_profile))
    full = np.zeros((2, 64, N), np.float32)
    for core in range(NCORES):
        b, qb = core // 4, core % 4
        full[b, :, qb * QBLK:(qb + 1) * QBLK] = res.results[core]["out"]
    out = full.reshape(2, 64, 64, 64)
    if _profile:
        return out, res
    return out


# revision 20
# speedup vs baseline: 1.1527x; 1.1527x over previous
"""Trainium2 Bass kernel for nn_BoundaryConditionedFusion.

Sharding: 8 cores = 2 batches x 4 query-blocks of 1024 pixels (16 image rows).
Each core computes k/v/b_weight for its full batch (cheap 1x1 convs) and
flash-style attention + proj + 3x3 local conv only for its query block.
All BatchNorms are folded into weights on the host; per-core inputs carry the
query-block-specific slices (with halo rows) so one SPMD program serves all
cores. Host reassembles the 8 [64, 1024] output slices.

Attention per (head, q-block): T = K^T Q computed k-major in [128, 512] PSUM
chunks (4 per group via PE row-tiling at tile_position (32g, 0) with q/k
replicated at partition offsets 0/32/64/96), exp over the 4-bank group in one
ScalarE op, then PV accumulated with an M=33 stationary (8 v dims + ones col
at partition 32) so out^T and the softmax denominator fall out of one matmul
chain. Softmax max-subtraction is skipped: logits are bounded (|T| < 9).
"""
import sys
import types
import numpy as np

C = 64
HEADS = 8
HD = 8
EPS = 1e-5
N = 4096
SCALE = HD ** -0.5
QBLK = 1024
NCORES = 8

# ---------------------------------------------------------------- compat shims


def _install_ntff_shim():
    try:
        from antenv import axon_hooks  # noqa: F401
        return
    except ImportError:
        pass
    try:
        from trn_agent_boot.trn_boot import _ntff_profile_via_ctypes
        hook = _ntff_profile_via_ctypes('/opt/axon/libaxon_pjrt.so')
    except Exception:
        return
    mod = types.ModuleType("antenv.axon_hooks")
    mod._hook = hook
    mod.get_axon_ntff_profile_hook = lambda: mod._hook
    mod.set_axon_ntff_profile_hook = lambda h: setattr(mod, "_hook", h)
    sys.modules["antenv.axon_hooks"] = mod
    import antenv
    antenv.axon_hooks = mod


def _fix_multiwait(raw: bytes) -> bytes:
    """This walrus build allows one sync-wait per instruction; hoist extras
    onto preceding same-engine NoOps."""
    import orjson
    m = orjson.loads(raw)

    def fix_block(insts):
        out = []
        for inst in insts:
            si = inst.get("sync_info") or {}
            waits = si.get("on_wait") or []
            if len(waits) > 1:
                for j, w in enumerate(waits[:-1]):
                    out.append({
                        "debug": inst.get("debug", 0),
                        "engine": inst["engine"],
                        "ins": [], "outs": [],
                        "name": f"{inst['name']}_w{j}",
                        "opcode": "NoOp",
                        "sync_info": {"on_update": [], "on_wait": [w]},
                    })
                inst = dict(inst)
                inst["sync_info"] = {"on_update": si.get("on_update") or [],
                                     "on_wait": [waits[-1]]}
            out.append(inst)
        return out

    def walk(o):
        if isinstance(o, dict):
            if isinstance(o.get("instructions"), list):
                o["instructions"] = fix_block(o["instructions"])
            for v in o.values():
                walk(v)
        elif isinstance(o, list):
            for v in o:
                walk(v)

    walk(m)
    return orjson.dumps(m)


# ---------------------------------------------------------------- host prep


def _resize_matrix(n_out, n_in):
    """Row matrix of jax.image.resize(..., 'bilinear', antialias=True)."""
    R = np.zeros((n_out, n_in), np.float64)
    scale = n_in / n_out
    for i in range(n_out):
        center = (i + 0.5) * scale - 0.5
        lo = int(np.floor(center - scale))
        hi = int(np.ceil(center + scale))
        ws, js = [], []
        for j in range(max(lo, 0), min(hi + 1, n_in)):
            w = max(0.0, 1.0 - abs(j - center) / scale)
            if w > 0:
                js.append(j)
                ws.append(w)
        ws = np.array(ws)
        ws /= ws.sum()
        for j, w in zip(js, ws):
            R[i, j] += w
    return R.astype(np.float32)


def _fold_bn(p):
    gamma, beta, mean, var = [np.asarray(t, np.float32) for t in p]
    s = gamma / np.sqrt(var + EPS)
    return s, beta - mean * s


def _prep_weights(params):
    pr = {}
    s_in, sh_in = _fold_bn(params['norm_in'])
    Wqkv = np.asarray(params['qkv_w'], np.float32)
    Wf = Wqkv * s_in[None, :]
    cqkv = Wqkv @ sh_in
    pr['wq'] = np.ascontiguousarray(Wf[:64].T).astype(np.float16)   # [128, 64]
    pr['wk'] = np.ascontiguousarray(Wf[64:128].T).astype(np.float16)
    pr['wv'] = np.ascontiguousarray(Wf[128:].T).astype(np.float16)
    pr['cq'] = cqkv[:64].reshape(64, 1).copy()
    pr['ck'] = cqkv[64:128].reshape(64, 1).copy()
    cv = cqkv[128:].copy()
    s_b, sh_b = _fold_bn(params['bproj_bn'])
    Wb = np.asarray(params['bproj_w'], np.float32)[:, 0] * s_b[:, None, None]
    pr['wb'] = np.ascontiguousarray(Wb.reshape(64, 9).T).astype(np.float16)  # [9, 64]
    pr['cb'] = (np.asarray(params['bproj_b'], np.float32) * s_b + sh_b).reshape(64, 1).copy()
    s_p, sh_p = _fold_bn(params['norm'])
    Wp = np.asarray(params['proj_w'], np.float32) * s_p[:, None]
    bp = np.asarray(params['proj_b'], np.float32) * s_p + sh_p
    pr['wp'] = np.ascontiguousarray(Wp.T).astype(np.float16)        # [64, 64]
    s_l, sh_l = _fold_bn(params['local_bn'])
    Wl = np.asarray(params['local_w'], np.float32) * s_l[:, None, None, None]
    bl = np.asarray(params['local_b'], np.float32) * s_l + sh_l
    pr['wl'] = np.ascontiguousarray(
        Wl.reshape(64, 128, 9).transpose(1, 2, 0).reshape(128, 9 * 64))
    pr['bf'] = (bl + bp + Wp @ cv).reshape(64, 1).copy()
    R = _resize_matrix(64, 128)
    pr['rT'] = np.ascontiguousarray(R.T)                  # [128, 64]
    expd = np.zeros((16, 128), np.float32)
    for i in range(16):
        expd[i, 8 * i:8 * i + 8] = 1.0
    pr['expd'] = expd
    Rpad = np.zeros((66, 134), np.float32)
    Rpad[1:65, 3:131] = R
    pr['rqT'] = []
    for qb in range(4):
        r0 = qb * 16
        # b_down rows r0-1..r0+16 over bm rows 2*r0-3..2*r0+34 (pad offset 1 / 3)
        Rq = Rpad[r0:r0 + 18, 2 * r0:2 * r0 + 38]         # [18, 38]
        pr['rqT'].append(np.ascontiguousarray(Rq.T))      # [38, 18]
    return pr


def _build_in_maps(f1, f2, bm, pr):
    f1 = np.asarray(f1, np.float32).reshape(2, 64, N)
    f2 = np.asarray(f2, np.float32).reshape(2, 64, N)
    bm = np.asarray(bm, np.float32).reshape(2, 128, 128)
    bmpad = np.zeros((2, 134, 128), np.float32)
    bmpad[:, 3:131, :] = bm
    maps = []
    for core in range(NCORES):
        b, qb = core // 4, core % 4
        r0 = qb * 16
        xq = np.zeros((128, 18, 64), np.float32)
        lo, hi = max(0, r0 - 1), min(64, r0 + 17)
        xq[:64, lo - (r0 - 1):hi - (r0 - 1), :] = f1[b].reshape(64, 64, 64)[:, lo:hi, :]
        xq[64:, lo - (r0 - 1):hi - (r0 - 1), :] = f2[b].reshape(64, 64, 64)[:, lo:hi, :]
        m = {
            "xall1": np.ascontiguousarray(f1[b]).astype(np.float16),
            "xall2": np.ascontiguousarray(f2[b]).astype(np.float16),
            "xq": np.ascontiguousarray(xq.reshape(128, 18 * 64)).astype(np.float16),
            "xqf": np.ascontiguousarray(xq.reshape(128, 18 * 64)),
            "bm": np.ascontiguousarray(bm[b]),
            "bmq": np.ascontiguousarray(bmpad[b, 2 * r0:2 * r0 + 38, :]),
            "rqT": pr['rqT'][qb],
        }
        for nm in ("wq", "wk", "wv", "cq", "ck", "wb", "cb", "wp", "wl", "bf", "rT",
                   "expd"):
            m[nm] = pr[nm]
        maps.append(m)
    return maps


# ---------------------------------------------------------------- device program


def _build_program():
    import concourse.bass as bass
    import concourse.mybir as mybir
    from concourse.tile import TileContext

    F32 = mybir.dt.float32
    F16 = mybir.dt.float16
    AF = mybir.ActivationFunctionType
    taps = [(dr, dc) for dr in (-1, 0, 1) for dc in (-1, 0, 1)]

    nc = bass.Bass()
    di = lambda nm, shp, dt=F32: nc.declare_dram_parameter(nm, shp, dt, isOutput=False)
    xall1 = di("xall1", [64, N], F16)
    xall2 = di("xall2", [64, N], F16)
    xq_d = di("xq", [128, 18 * 64], F16)
    xqf_d = di("xqf", [128, 18 * 64])
    bm_d = di("bm", [128, 128])
    bmq_d = di("bmq", [38, 128])
    rT_d = di("rT", [128, 64])
    rqT_d = di("rqT", [38, 18])
    wq_d = di("wq", [128, 64], F16)
    wk_d = di("wk", [128, 64], F16)
    wv_d = di("wv", [128, 64], F16)
    cq_d = di("cq", [64, 1])
    ck_d = di("ck", [64, 1])
    wb_d = di("wb", [9, 64], F16)
    cb_d = di("cb", [64, 1])
    wp_d = di("wp", [64, 64], F16)
    wl_d = di("wl", [128, 9 * 64])
    bf_d = di("bf", [64, 1])
    expd_d = di("expd", [16, 128])
    out_d = nc.declare_dram_parameter("out", [64, QBLK], F32, isOutput=True)

    with TileContext(nc) as tc:
        with (
            tc.tile_pool(name="w", bufs=1) as wpool,
            tc.tile_pool(name="big", bufs=1) as big,
            tc.tile_pool(name="asm", bufs=2) as asm,
        ):
            wp_s = wpool.tile([128, 64], F16)
            nc.sync.dma_start(out=wp_s[0:64, :], in_=wp_d[:])
            nc.sync.dma_start(out=wp_s[64:128, :], in_=wp_d[:])
            wl_s = wpool.tile([128, 9 * 64], F32); nc.sync.dma_start(out=wl_s[:], in_=wl_d[:])
            bf_s = wpool.tile([64, 1], F32);  nc.sync.dma_start(out=bf_s[:], in_=bf_d[:])
            expd_s = wpool.tile([16, 128], F32)
            nc.sync.dma_start(out=expd_s[:], in_=expd_d[:])

            xq_pad = big.tile([128, 18 * 66], F16)
            xq3 = xq_pad[:].rearrange("p (r c) -> p r c", c=66)
            nc.vector.memset(xq_pad[:].rearrange("p (r c) -> p (r) c", c=66)[:, :, 65:66], 0.0)
            nc.vector.memset(xq3[:, :, 0:1], 0.0)
            nc.sync.dma_start(out=xq3[:, :, 1:65],
                              in_=xq_d[:].rearrange("p (r c) -> p r c", c=64))
            xq_padF = big.tile([128, 18 * 66], F32)
            xqF3 = xq_padF[:].rearrange("p (r c) -> p r c", c=66)
            nc.vector.memset(xqF3[:, :, 0:1], 0.0)
            nc.vector.memset(xqF3[:, :, 65:66], 0.0)
            nc.sync.dma_start(out=xqF3[:, :, 1:65],
                              in_=xqf_d[:].rearrange("p (r c) -> p r c", c=64))
            k16 = big.tile([64, N], F16)
            q16 = big.tile([64, QBLK], F16)
            v_all = big.tile([128, 32 * 264], F16)
            shifts = []
            for t, (dr, dc) in enumerate(taps):
                sh = big.tile([128, QBLK], F32, tag=f"sh{t}")
                eng = nc.sync if t % 2 == 0 else nc.gpsimd
                eng.dma_start(out=sh[:],
                              in_=xqF3[:, 1 + dr:17 + dr, 1 + dc:65 + dc])
                shifts.append(sh)

            # ---------------- fringe
            with (
                tc.tile_pool(name="fr", bufs=1) as fr,
                tc.tile_pool(name="ftmp", bufs=2) as ftmp,
                tc.tile_pool(name="fps", bufs=2, space="PSUM") as fps,
            ):
                wq_s = fr.tile([128, 64], F16); nc.sync.dma_start(out=wq_s[:], in_=wq_d[:])
                wk_s = fr.tile([128, 64], F16); nc.sync.dma_start(out=wk_s[:], in_=wk_d[:])
                wv_s = fr.tile([128, 64], F16); nc.sync.dma_start(out=wv_s[:], in_=wv_d[:])
                cq_s = fr.tile([64, 1], F32);  nc.sync.dma_start(out=cq_s[:], in_=cq_d[:])
                ck_s = fr.tile([64, 1], F32);  nc.sync.dma_start(out=ck_s[:], in_=ck_d[:])
                wb_s = fr.tile([9, 64], F16);  nc.gpsimd.dma_start(out=wb_s[:], in_=wb_d[:])
                cb_s = fr.tile([64, 1], F32);  nc.gpsimd.dma_start(out=cb_s[:], in_=cb_d[:])
                rT_s = fr.tile([128, 64], F32); nc.gpsimd.dma_start(out=rT_s[:], in_=rT_d[:])
                rqT_s = fr.tile([38, 18], F32); nc.gpsimd.dma_start(out=rqT_s[:], in_=rqT_d[:])
                bm_s = fr.tile([128, 128], F32); nc.gpsimd.dma_start(out=bm_s[:], in_=bm_d[:])
                bmq_s = fr.tile([38, 128], F32); nc.gpsimd.dma_start(out=bmq_s[:], in_=bmq_d[:])

                x_cm = fr.tile([128, N], F16)
                nc.sync.dma_start(out=x_cm[0:64, :], in_=xall1[:])
                nc.sync.dma_start(out=x_cm[64:128, :], in_=xall2[:])
                xq_cm = fr.tile([128, QBLK], F16)
                nc.sync.dma_start(out=xq_cm[:], in_=xq_d[:, 64:64 + QBLK])

                # bilinear resize (fp32 PE), outputs cast to fp16
                psu = fps.tile([128, 64], F32, tag="fp")
                nc.tensor.matmul(psu[:], bm_s[:], rT_s[:], start=True, stop=True)
                u_sb = ftmp.tile([128, 64], F32, tag="t_r")
                nc.vector.tensor_copy(u_sb[:], psu[:])
                psbd = fps.tile([64, 64], F32, tag="fp")
                nc.tensor.matmul(psbd[:], u_sb[:], rT_s[:], start=True, stop=True)
                bdown = fr.tile([64, 64], F16)
                nc.vector.tensor_copy(bdown[:], psbd[:])
                pstq = fps.tile([128, 18], F32, tag="fp")
                nc.tensor.matmul(pstq[:], bmq_s[:], rqT_s[:], start=True, stop=True)
                tq_sb = ftmp.tile([128, 18], F32, tag="t_r")
                nc.vector.tensor_copy(tq_sb[:], pstq[:])
                psbdq = fps.tile([18, 64], F32, tag="fp")
                nc.tensor.matmul(psbdq[:], tq_sb[:], rT_s[:], start=True, stop=True)
                bdownq = fr.tile([18, 64], F16)
                nc.vector.tensor_copy(bdownq[:], psbdq[:])

                # v pixel-major (overlaps the patch DMAs on PE)
                onesv = v_all[:].rearrange("p (ch d) -> p ch d", d=33)
                nc.vector.memset(onesv[:, :, 32:33], 1.0)
                for c in range(32):
                    psv = fps.tile([128, 64], F32, tag="fp")
                    nc.tensor.matmul(psv[:], x_cm[:, c * 128:(c + 1) * 128], wv_s[:],
                                     start=True, stop=True)
                    dstv = v_all[:, c * 264:(c + 1) * 264].rearrange(
                        "p (h d) -> p h d", d=33)
                    nc.vector.tensor_copy(dstv[:, :, 0:8],
                                          psv[:].rearrange("p (h d) -> p h d", d=8))

                patches = fr.tile([9, N], F16)
                nc.vector.memset(patches[:], 0.0)
                patches_q = fr.tile([9, QBLK], F16)
                nc.vector.memset(patches_q[:], 0.0)
                for t, (dr, dc) in enumerate(taps):
                    rs, re = max(0, -dr), min(64, 64 - dr)
                    cs, ce = max(0, -dc), min(64, 64 - dc)
                    eng = nc.sync if t % 2 == 0 else nc.gpsimd
                    dstp = patches[t:t + 1, :].rearrange("p (r c) -> p r c", c=64)
                    eng.dma_start(out=dstp[:, rs:re, cs:ce],
                                  in_=bdown[rs + dr:re + dr, cs + dc:ce + dc])
                    dstpq = patches_q[t:t + 1, :].rearrange("p (r c) -> p r c", c=64)
                    eng.dma_start(out=dstpq[:, 0:16, cs:ce],
                                  in_=bdownq[1 + dr:17 + dr, cs + dc:ce + dc])

                bwp = fr.tile([64, N], F32)
                for blk in range(8):
                    psb = fps.tile([64, 512], F32, tag="fp")
                    nc.tensor.matmul(psb[:], wb_s[:],
                                     patches[:, blk * 512:(blk + 1) * 512],
                                     start=True, stop=True)
                    sg = ftmp.tile([64, 512], F32, tag="sg")
                    nc.scalar.activation(sg[:], psb[:], AF.Sigmoid, bias=cb_s[:])
                    nc.vector.tensor_scalar_add(bwp[:, blk * 512:(blk + 1) * 512],
                                                sg[:], 1.0)
                bwq = fr.tile([64, QBLK], F32)
                for blk in range(2):
                    psb = fps.tile([64, 512], F32, tag="fp")
                    nc.tensor.matmul(psb[:], wb_s[:],
                                     patches_q[:, blk * 512:(blk + 1) * 512],
                                     start=True, stop=True)
                    sg = ftmp.tile([64, 512], F32, tag="sg")
                    nc.scalar.activation(sg[:], psb[:], AF.Sigmoid, bias=cb_s[:])
                    nc.vector.tensor_scalar(bwq[:, blk * 512:(blk + 1) * 512], sg[:],
                                            1.0, SCALE, mybir.AluOpType.add,
                                            mybir.AluOpType.mult)

                # k16 / q16
                for blk in range(8):
                    psk = fps.tile([64, 512], F32, tag="fp")
                    nc.tensor.matmul(psk[:], wk_s[:],
                                     x_cm[:, blk * 512:(blk + 1) * 512],
                                     start=True, stop=True)
                    kt = ftmp.tile([64, 512], F32, tag="kt")
                    nc.vector.tensor_scalar_add(kt[:], psk[:], ck_s[:])
                    nc.vector.tensor_mul(k16[:, blk * 512:(blk + 1) * 512], kt[:],
                                         bwp[:, blk * 512:(blk + 1) * 512])
                for blk in range(2):
                    psq = fps.tile([64, 512], F32, tag="fp")
                    nc.tensor.matmul(psq[:], wq_s[:],
                                     xq_cm[:, blk * 512:(blk + 1) * 512],
                                     start=True, stop=True)
                    qt = ftmp.tile([64, 512], F32, tag="kt")
                    nc.vector.tensor_scalar_add(qt[:], psq[:], cq_s[:])
                    nc.vector.tensor_mul(q16[:, blk * 512:(blk + 1) * 512], qt[:],
                                         bwq[:, blk * 512:(blk + 1) * 512])


            # ---------------- attention: flat pipeline over all (h, qb, group)
            GROUPS = [4, 3, 4, 3, 4, 3, 4, 3, 2, 2]
            GSTART = [sum(GROUPS[:i]) for i in range(len(GROUPS))]
            NG = len(GROUPS)
            with (
                tc.tile_pool(name="arep", bufs=2) as arep,
                tc.tile_pool(name="aE", bufs=3) as aE,
                tc.tile_pool(name="apsA", bufs=1, space="PSUM") as apsA,
                tc.tile_pool(name="apsB", bufs=1, space="PSUM") as apsB,
                tc.tile_pool(name="apsO", bufs=1, space="PSUM") as apsO,
            ):
                U_all = asm.tile([128, 512], F32, tag="uall")
                den_all = asm.tile([16, 512], F32, tag="dall")
                reps = {}
                psOs = {}
                Es = {}

                def load_head(h):
                    k_rep = arep.tile([128, N], F16, tag="krep")
                    q_rep = arep.tile([128, QBLK], F16, tag="qrep")
                    for g in range(4):
                        nc.gpsimd.dma_start(out=k_rep[32 * g:32 * g + 8, :],
                                            in_=k16[8 * h:8 * h + 8, :])
                        nc.gpsimd.dma_start(out=q_rep[32 * g:32 * g + 8, :],
                                            in_=q16[8 * h:8 * h + 8, :])
                    reps[h] = (k_rep, q_rep)

                def emit_m1(idx, h, qb, gi):
                    k_rep, q_rep = reps[h]
                    ng, st = GROUPS[gi], GSTART[gi]
                    pool = apsA if gi % 2 == 0 else apsB
                    psT = pool.tile([128, 512 * ng], F32,
                                    tag="psTA" if gi % 2 == 0 else "psTB")
                    for g in range(ng):
                        c = st + g
                        nc.tensor.matmul(
                            psT[:, g * 512:(g + 1) * 512],
                            k_rep[32 * g:32 * g + 8, c * 128:(c + 1) * 128],
                            q_rep[32 * g:32 * g + 8, qb * 512:(qb + 1) * 512],
                            start=True, stop=True,
                            tile_position=(32 * g, 0))
                    E = aE.tile([128, 512 * 4], F16, tag="E")
                    nc.scalar.activation(E[:, :512 * ng], psT[:], AF.Exp)
                    Es[idx] = E

                def emit_m2(idx, h, qb, gi):
                    pk = (h, qb)
                    if pk not in psOs:
                        psOs[pk] = apsO.tile([128, 512], F32, tag="psO", name="psO")
                    psO = psOs[pk]
                    ng, st = GROUPS[gi], GSTART[gi]
                    E = Es.pop(idx)
                    for g in range(ng):
                        c = st + g
                        nc.tensor.matmul(
                            psO[0:33, :],
                            v_all[:, c * 264 + 33 * h:c * 264 + 33 * h + 33],
                            E[:, g * 512:(g + 1) * 512],
                            start=(c == 0), stop=(c == 31))
                    if gi == NG - 1:
                        i = qb * 8 + h
                        stage = asm.tile([33, 512], F32, tag="stage")
                        nc.vector.tensor_copy(stage[:], psO[0:33, :])
                        nc.sync.dma_start(out=den_all[i:i + 1, :],
                                          in_=stage[32:33, :])
                        nc.sync.dma_start(out=U_all[8 * i:8 * i + 8, :],
                                          in_=stage[0:8, :])
                        del psOs[pk]

                sched = [(h, qb, gi) for h in range(HEADS) for qb in range(2)
                         for gi in range(NG)]
                load_head(0)
                for idx, (h, qb, gi) in enumerate(sched):
                    if qb == 1 and gi == 0 and h + 1 < HEADS:
                        load_head(h + 1)
                    emit_m1(idx, h, qb, gi)
                    if idx >= 2:
                        emit_m2(idx - 2, *sched[idx - 2])
                emit_m2(len(sched) - 2, *sched[-2])
                emit_m2(len(sched) - 1, *sched[-1])

            # ---------------- tail: local conv overlaps normalize, then proj+relu
            with (
                tc.tile_pool(name="fin", bufs=2) as fin,
                tc.tile_pool(name="ops", bufs=2, space="PSUM") as ops,
            ):
                psfs = []
                for qb in range(2):
                    psf = ops.tile([64, 512], F32, tag="psf")
                    for t in range(9):
                        nc.tensor.matmul(psf[:], wl_s[:, t * 64:(t + 1) * 64],
                                         shifts[t][:, qb * 512:(qb + 1) * 512],
                                         start=(t == 0), stop=False)
                    psfs.append(psf)
                rden_all = asm.tile([16, 512], F32, tag="rdall")
                nc.vector.reciprocal(rden_all[:], den_all[:])
                psB = ops.tile([128, 512], F32, tag="psB")
                nc.tensor.matmul(psB[:], expd_s[:], rden_all[:], start=True, stop=True)
                att16 = asm.tile([128, 512], F16, tag="att16")
                nc.vector.tensor_mul(att16[:], U_all[:], psB[:])
                for qb in range(2):
                    nc.tensor.matmul(psfs[qb][:], wp_s[64 * qb:64 * qb + 64, :],
                                     att16[64 * qb:64 * qb + 64, :],
                                     start=False, stop=True,
                                     tile_position=(64 * qb, 0))
                    o_sb = fin.tile([64, 512], F32, tag="osb")
                    nc.scalar.activation(o_sb[:], psfs[qb][:], AF.Relu, bias=bf_s[:])
                    nc.sync.dma_start(out=out_d[:, qb * 512:(qb + 1) * 512],
                                      in_=o_sb[:])

    return nc


_CACHED = {}


def kernel(f1, f2, boundary_map, params, _profile=False):
    _install_ntff_shim()
    from concourse.bass_utils import run_bass_kernel_spmd

    if "nc" not in _CACHED:
        nc = _build_program()
        orig = nc.to_json_bytes
        nc.to_json_bytes = lambda *a, **k: _fix_multiwait(orig(*a, **k))
        _CACHED["nc"] = nc
    nc = _CACHED["nc"]

    pr = _prep_weights(params)
    in_maps = _build_in_maps(f1, f2, boundary_map, pr)
    res = run_bass_kernel_spmd(nc, in_maps, core_ids=list(range(NCORES)),
                               trace=bool(_profile))
    full = np.zeros((2, 64, N), np.float32)
    for core in range(NCORES):
        b, qb = core // 4, core % 4
        full[b, :, qb * QBLK:(qb + 1) * QBLK] = res.results[core]["out"]
    out = full.reshape(2, 64, 64, 64)
    if _profile:
        return out, res
    return out
